# revision 5
# baseline (speedup 1.0000x reference)
"""Trainium2 Bass kernel for sparse-projection + WTA top-k masking.

Computes out = topk_mask_32(input @ W.T) where W [10240, 512] is built from
per-row COO entries (weight_vals/weight_idx, duplicates accumulate).

Strategy (hardcoded for B=4096, F=512, O=10240, K=32, 8 cores):
  - Shard the OUTPUT dim across cores (O-shard): each core computes the full
    batch against a 1280-column slice of W.T.  Per-core DMA: 4MB inT (fp16,
    replicated) + 1.25MB wt slice in, 10MB scores out = ~15.6MB total, well
    under the ~70us tensor-engine floor.
  - Device (SPMD x8): fp16 matmul (1 col/cycle, same rate as fp32r, half the
    SBUF/DMA).  x[o,b] = wtT.T @ inT, PSUM accumulated over 4 k-tiles.
    Loop: o-tile > b-half > k > b; the stationary operand is reused across 4
    consecutive MMs (the fused LDWEIGHTS serializes with its MM, so reuse
    amortizes it), and each b-half owns a 4-bank PSUM tile so the two halves
    ping-pong and evictions never stall the PE.  Each [128, 2048] PSUM tile
    is evicted by ONE wide ACT/DVE copy (engines alternate; each ends up
    ~20us busy vs the PE's ~73us) straight into an fp16 staging tile that is
    DMA'd out with 4KB partition lines.  Engine roles: tensor=MM,
    scalar/vector=wt preload + evictions, sync=input stream + score stores.
    No on-device top-k: the old max8/find_index8 pipeline put ~100us on the
    DVE and was the bottleneck; shipping fp16 scores costs ~29us of
    overlapped DMA instead.
  - Host: top-64 candidates per row from the fp16 scores (argpartition), then
    EXACT recompute of those 64 via the 32-entry COO rows (vectorized gather,
    no GEMM), exact top-32 of the 64, scatter.  fp16 worst-case score error
    (~0.02) cannot demote a true top-32 element past approx rank 64 (the
    rank-32..96 value spread is ~2), so no margin rescue pass is needed and
    output values are exact fp32.
"""

import numpy as np
import concourse.bacc as bacc
import concourse.bass as bass
import concourse.tile as tile
import concourse.mybir as mybir
from concourse.bass_utils import run_bass_kernel_spmd

F32 = mybir.dt.float32
F16 = mybir.dt.float16

B = 4096          # batch
F = 512           # in_features
O = 10240         # out_features
TOPK = 32
NCORES = 8
OL = O // NCORES  # 1280 output cols per core
OT = OL // 128    # 10 o-tiles per core
KT = F // 128     # 4 k-tiles
NB = 512          # b-chunk width (moving operand / one PSUM bank fp32)
BCH = B // NB     # 8 b-chunks
NCAND = 64        # host-side candidate count per row


def build_program() -> bass.Bass:
    nc = bacc.Bacc()
    inT = nc.declare_dram_parameter("inT", [F, B], F16, isOutput=False)
    wt = nc.declare_dram_parameter("wt", [F, OL], F16, isOutput=False)
    x_d = nc.declare_dram_parameter("x", [OL, B], F16, isOutput=True)

    with tile.TileContext(nc) as tc:
        with (
            tc.tile_pool(name="insb", bufs=1) as inpool,
            tc.tile_pool(name="wtsb", bufs=1) as wtpool,
            tc.tile_pool(name="psum", bufs=2, space=bass.MemorySpace.PSUM) as pspool,
            tc.tile_pool(name="xout", bufs=4) as xpool,
        ):
            insb = [inpool.tile([128, B], F16, name=f"in{k}", tag=f"in{k}")
                    for k in range(KT)]
            wtsb = [wtpool.tile([128, OL], F16, name=f"wt{k}", tag=f"wt{k}")
                    for k in range(KT)]

            # only sync+scalar are HWDGE engines: weights stream on scalar
            # (idle until its first eviction), input on sync in 1024-wide
            # (2KB-line) b-superchunks.  The MM loop is b-half OUTER, so the
            # whole first half of the kernel touches only input cols 0:2048 —
            # k-major load order then makes the kernel DMA-paced only through
            # o-tile 0 (~1.4us) instead of stalling mid-kernel.
            for k in range(KT):
                nc.scalar.dma_start(wtsb[k][:], wt[k * 128:(k + 1) * 128, :])
            for bh in range(2):
                for k in range(KT):
                    for b2 in (2 * bh, 2 * bh + 1):
                        nc.sync.dma_start(
                            insb[k][:, b2 * 1024:(b2 + 1) * 1024],
                            inT[k * 128:(k + 1) * 128, b2 * 1024:(b2 + 1) * 1024])

            for bh in range(2):
                for ot in range(OT):
                    ps = pspool.tile([128, 4 * NB], F32, name="ps", tag="ps")
                    for k in range(KT):
                        for j in range(4):
                            b = bh * 4 + j
                            nc.tensor.matmul(
                                ps[:, j * NB:(j + 1) * NB],
                                wtsb[k][:, ot * 128:(ot + 1) * 128],
                                insb[k][:, b * NB:(b + 1) * NB],
                                start=(k == 0),
                                stop=(k == KT - 1),
                            )
                    xh = xpool.tile([128, 4 * NB], F16, name="xh", tag="xh")
                    # evict halves on ACT and DVE in parallel (~1.2us each);
                    # the last one adds only ~1.8us of tail after the last MM
                    nc.scalar.copy(xh[:, 0:2 * NB], ps[:, 0:2 * NB])
                    nc.vector.tensor_copy(xh[:, 2 * NB:4 * NB], ps[:, 2 * NB:4 * NB])
                    nc.sync.dma_start(
                        x_d[ot * 128:(ot + 1) * 128,
                            bh * 2048:(bh + 1) * 2048],
                        xh[:])
    nc.compile()
    return nc


_NC = None


def _get_program() -> bass.Bass:
    global _NC
    if _NC is None:
        _NC = build_program()
    return _NC


# host-side context for gather_output's exact candidate recompute
_CTX = {}


def prepare_in_maps(input, weight_vals, weight_idx):
    input = np.ascontiguousarray(np.asarray(input, dtype=np.float32))
    weight_vals = np.asarray(weight_vals, dtype=np.float32)
    weight_idx = np.asarray(weight_idx).astype(np.int64)

    # Dense W on host (COO duplicates add), transposed + fp16 for the device.
    W = np.zeros((O, F), dtype=np.float32)
    np.add.at(W, (np.arange(O)[:, None], weight_idx), weight_vals)
    WT16 = np.ascontiguousarray(W.T.astype(np.float16))     # [F, O]
    inT16 = np.ascontiguousarray(input.T.astype(np.float16))  # [F, B]

    _CTX["input"] = input
    _CTX["weight_vals"] = weight_vals
    _CTX["weight_idx"] = weight_idx

    return [
        {"inT": inT16, "wt": np.ascontiguousarray(WT16[:, c * OL:(c + 1) * OL])}
        for c in range(NCORES)
    ]


def gather_output(results) -> np.ndarray:
    input = _CTX["input"]
    weight_vals = _CTX["weight_vals"]
    weight_idx = _CTX["weight_idx"]

    X = np.concatenate(
        [np.asarray(results[c]["x"]) for c in range(NCORES)], axis=0)  # [O, B]
    S = X.T.astype(np.float32)                                         # [B, O]

    # approx top-64 per row, then exact recompute of just those candidates
    # via the 32-entry COO rows (sum_p vals[o,p] * input[b, idx[o,p]])
    cand = np.argpartition(-S, NCAND - 1, axis=1)[:, :NCAND]   # [B, 64]
    rows = np.arange(B)[:, None]
    wi = weight_idx[cand]                                      # [B, 64, 32]
    wv = weight_vals[cand].astype(np.float64)                  # [B, 64, 32]
    xg = input[rows[:, :, None], wi]                           # [B, 64, 32]
    exact = (wv * xg).sum(axis=2)                              # [B, 64] f64

    # exact top-32 of the 64 (desc value, ties by lower column like top_k)
    order = np.lexsort((cand, -exact), axis=1)[:, :TOPK]
    g32 = np.take_along_axis(cand, order, axis=1)
    v32 = np.take_along_axis(exact, order, axis=1).astype(np.float32)

    out = np.zeros((B, O), dtype=np.float32)
    out[rows, g32] = v32
    return out


def kernel(input, weight_vals, weight_idx):
    in_maps = prepare_in_maps(input, weight_vals, weight_idx)
    res = run_bass_kernel_spmd(_get_program(), in_maps, list(range(NCORES)))
    return gather_output(res.results)


# revision 8
# speedup vs baseline: 1.0633x; 1.0633x over previous
"""Trainium2 Bass kernel for sparse-projection + WTA top-k masking.

Computes out = topk_mask_32(input @ W.T) where W [10240, 512] is built from
per-row COO entries (weight_vals/weight_idx, duplicates accumulate).

Strategy (hardcoded for B=4096, F=512, O=10240, K=32, 8 cores):
  - Shard the OUTPUT dim across cores (O-shard): each core computes the full
    batch against a 1280-column slice of W.T.  Per-core DMA: 4MB inT (fp16,
    replicated) + 1.25MB wt slice in, 10MB scores out = ~15.6MB total, well
    under the ~70us tensor-engine floor.
  - Device (SPMD x8): fp16 matmul (1 col/cycle, same rate as fp32r, half the
    SBUF/DMA).  x[o,b] = wtT.T @ inT, PSUM accumulated over 4 k-tiles.
    Loop: o-tile > b-half > k > b; the stationary operand is reused across 4
    consecutive MMs (the fused LDWEIGHTS serializes with its MM, so reuse
    amortizes it), and each b-half owns a 4-bank PSUM tile so the two halves
    ping-pong and evictions never stall the PE.  Each [128, 2048] PSUM tile
    is evicted by ONE wide ACT/DVE copy (engines alternate; each ends up
    ~20us busy vs the PE's ~73us) straight into an fp16 staging tile that is
    DMA'd out with 4KB partition lines.  Engine roles: tensor=MM,
    scalar/vector=wt preload + evictions, sync=input stream + score stores.
    No on-device top-k: the old max8/find_index8 pipeline put ~100us on the
    DVE and was the bottleneck; shipping fp16 scores costs ~29us of
    overlapped DMA instead.
  - Host: top-64 candidates per row from the fp16 scores (argpartition), then
    EXACT recompute of those 64 via the 32-entry COO rows (vectorized gather,
    no GEMM), exact top-32 of the 64, scatter.  fp16 worst-case score error
    (~0.02) cannot demote a true top-32 element past approx rank 64 (the
    rank-32..96 value spread is ~2), so no margin rescue pass is needed and
    output values are exact fp32.
"""

import numpy as np
import concourse.bacc as bacc
import concourse.bass as bass
import concourse.tile as tile
import concourse.mybir as mybir
from concourse.bass_utils import run_bass_kernel_spmd

F32 = mybir.dt.float32
F16 = mybir.dt.float16

B = 4096          # batch
F = 512           # in_features
O = 10240         # out_features
TOPK = 32
NCORES = 8
OL = O // NCORES  # 1280 output cols per core
OT = OL // 128    # 10 o-tiles per core
KT = F // 128     # 4 k-tiles
NB = 512          # b-chunk width (moving operand / one PSUM bank fp32)
BCH = B // NB     # 8 b-chunks
NCAND = 64        # host-side candidate count per row


def build_program() -> bass.Bass:
    nc = bacc.Bacc()
    inT = nc.declare_dram_parameter("inT", [F, B], F16, isOutput=False)
    wt = nc.declare_dram_parameter("wt", [F, OL], F16, isOutput=False)
    x_d = nc.declare_dram_parameter("x", [OL, B], F16, isOutput=True)

    with tile.TileContext(nc) as tc:
        with (
            tc.tile_pool(name="insb", bufs=1) as inpool,
            tc.tile_pool(name="wtsb", bufs=1) as wtpool,
            tc.tile_pool(name="psum", bufs=4, space=bass.MemorySpace.PSUM) as pspool,
            tc.tile_pool(name="xout", bufs=6) as xpool,
        ):
            insb = [inpool.tile([128, B], F16, name=f"in{k}", tag=f"in{k}")
                    for k in range(KT)]
            wtsb = [wtpool.tile([128, OL], F16, name=f"wt{k}", tag=f"wt{k}")
                    for k in range(KT)]

            # only sync+scalar are HWDGE engines: weights stream on scalar
            # (idle until its first eviction), input on sync in 1024-wide
            # (2KB-line) b-superchunks.  The MM loop runs QUARTER passes (one
            # 1024-wide b-superchunk across all 10 o-tiles, ~19us of PE work
            # each), so pass 0 only needs wt + superchunk 0 = 2.25MB before
            # the PE saturates, and the input stream (strictly in
            # consumption order) stays ~one pass ahead thereafter.
            for k in range(KT):
                nc.scalar.dma_start(wtsb[k][:], wt[k * 128:(k + 1) * 128, :])
            for sc in range(B // 1024):
                for k in range(KT):
                    nc.sync.dma_start(
                        insb[k][:, sc * 1024:(sc + 1) * 1024],
                        inT[k * 128:(k + 1) * 128, sc * 1024:(sc + 1) * 1024])

            for sc in range(B // 1024):
                for ot in range(OT):
                    ps = pspool.tile([128, 2 * NB], F32, name="ps", tag="ps")
                    for k in range(KT):
                        for j in range(2):
                            b = 2 * sc + j
                            nc.tensor.matmul(
                                ps[:, j * NB:(j + 1) * NB],
                                wtsb[k][:, ot * 128:(ot + 1) * 128],
                                insb[k][:, b * NB:(b + 1) * NB],
                                start=(k == 0),
                                stop=(k == KT - 1),
                            )
                    xh = xpool.tile([128, 2 * NB], F16, name="xh", tag="xh")
                    # alternate eviction engines by o-tile parity; each is
                    # ~1.2us busy per 1.94us of PE cadence across 2 engines
                    if ot % 2 == 0:
                        nc.scalar.copy(xh[:], ps[:])
                    else:
                        nc.vector.tensor_copy(xh[:], ps[:])
                    nc.sync.dma_start(
                        x_d[ot * 128:(ot + 1) * 128,
                            sc * 1024:(sc + 1) * 1024],
                        xh[:])
    nc.compile()
    return nc


_NC = None


def _get_program() -> bass.Bass:
    global _NC
    if _NC is None:
        _NC = build_program()
    return _NC


# host-side context for gather_output's exact candidate recompute
_CTX = {}


def prepare_in_maps(input, weight_vals, weight_idx):
    input = np.ascontiguousarray(np.asarray(input, dtype=np.float32))
    weight_vals = np.asarray(weight_vals, dtype=np.float32)
    weight_idx = np.asarray(weight_idx).astype(np.int64)

    # Dense W on host (COO duplicates add), transposed + fp16 for the device.
    W = np.zeros((O, F), dtype=np.float32)
    np.add.at(W, (np.arange(O)[:, None], weight_idx), weight_vals)
    WT16 = np.ascontiguousarray(W.T.astype(np.float16))     # [F, O]
    inT16 = np.ascontiguousarray(input.T.astype(np.float16))  # [F, B]

    _CTX["input"] = input
    _CTX["weight_vals"] = weight_vals
    _CTX["weight_idx"] = weight_idx

    return [
        {"inT": inT16, "wt": np.ascontiguousarray(WT16[:, c * OL:(c + 1) * OL])}
        for c in range(NCORES)
    ]


def gather_output(results) -> np.ndarray:
    input = _CTX["input"]
    weight_vals = _CTX["weight_vals"]
    weight_idx = _CTX["weight_idx"]

    X = np.concatenate(
        [np.asarray(results[c]["x"]) for c in range(NCORES)], axis=0)  # [O, B]
    S = X.T.astype(np.float32)                                         # [B, O]

    # approx top-64 per row, then exact recompute of just those candidates
    # via the 32-entry COO rows (sum_p vals[o,p] * input[b, idx[o,p]])
    cand = np.argpartition(-S, NCAND - 1, axis=1)[:, :NCAND]   # [B, 64]
    rows = np.arange(B)[:, None]
    wi = weight_idx[cand]                                      # [B, 64, 32]
    wv = weight_vals[cand].astype(np.float64)                  # [B, 64, 32]
    xg = input[rows[:, :, None], wi]                           # [B, 64, 32]
    exact = (wv * xg).sum(axis=2)                              # [B, 64] f64

    # exact top-32 of the 64 (desc value, ties by lower column like top_k)
    order = np.lexsort((cand, -exact), axis=1)[:, :TOPK]
    g32 = np.take_along_axis(cand, order, axis=1)
    v32 = np.take_along_axis(exact, order, axis=1).astype(np.float32)

    out = np.zeros((B, O), dtype=np.float32)
    out[rows, g32] = v32
    return out


def kernel(input, weight_vals, weight_idx):
    in_maps = prepare_in_maps(input, weight_vals, weight_idx)
    res = run_bass_kernel_spmd(_get_program(), in_maps, list(range(NCORES)))
    return gather_output(res.results)
